# revision 9
# baseline (speedup 1.0000x reference)
"""Trainium2 Bass kernel for nn_NeuralAttention (dense transformer block:
QKV projection + RoPE + softmax attention + output projection).

Sharding: 8 heads -> 8 NeuronCores (tensor parallel, Megatron-style).
Each core computes one head end-to-end from the full input x and produces a
partial output y_h = softmax((q_h k_h^T)/8) v_h @ wo[:, h].T of shape
[4096, 512]; the host sums the 8 partials.

All matmul operands are float32r (tf32-like, ~1e-4 relative error, 1 col/cycle
on the PE). Every contraction is padded to K=128: the TRN2 HAM clock gate
only counts the PE "busy" when the full array is active, so K=64 matmuls run
at 1.2 GHz forever while K=128 runs at 2.4 GHz. Padding the contraction with
zeros (producer side) or multiplying garbage rows by zero weights (consumer
side) doubles the clock at zero cycle cost.

Per-core pipeline (head h):
  xT [512, 4096] (host-transposed x) -> q|rot(q) and k|rot(k) concat-M
  projections -> RoPE via DVE mult with [cos;sin] + PE fold with
  [I;I | 0] -> q'T/k'T [128, 4096] (rows 64-127 zero)
  S.T chunk [128(tk), 512(q)] = kT2_chunk.T @ qT2 (K=128, hi half zeros)
  A.T = exp(S.T/8)  (ACT, PSUM->SBUF, f32r)
  O.T [128, q] += va2_chunk.T @ A.T  (va2 cols 64-127 = 1.0 -> rows 64-127 of
  O.T = softmax denominators), accumulated in PSUM over 32 chunks
  y qchunk [128, 512] = (OT2_chunk.T @ wo2) * recip(sums)  (wo2 rows 64-127
  zero), overlapped with the next q-quarter's attention.
"""

import numpy as np

import concourse.bacc as bacc
import concourse.tile as tile
from concourse import mybir
from concourse.bass import ds, ts
from concourse.bass_utils import run_bass_kernel_spmd

F32 = mybir.dt.float32
F32R = mybir.dt.float32r
EXP = mybir.ActivationFunctionType.Exp

T = 4096
HIDDEN = 512
N_HEADS = 8
HD = 64
N_CORES = 8
NBLK = T // 512
ROPE_BASE = 10000.0

_CACHE = {}


def _fold(nc, pf, istk2, qT2, kT2, pending):
    tq0, tk0, b0 = pending.pop(0)
    pfq = pf.tile([128, 512], F32, tag="pf", name="pfq")
    nc.tensor.matmul(pfq[:], istk2[:], tq0[:], start=True, stop=True)
    nc.scalar.copy(qT2[:, ts(b0, 512)], pfq[:].bitcast(F32))
    pfk = pf.tile([128, 512], F32, tag="pf", name="pfk")
    nc.tensor.matmul(pfk[:], istk2[:], tk0[:], start=True, stop=True)
    nc.scalar.copy(kT2[:, ts(b0, 512)], pfk[:].bitcast(F32))


def _build():
    nc = bacc.Bacc("TRN2", target_bir_lowering=False, debug=False,
                   num_devices=N_CORES)

    xT_d = nc.dram_tensor("xT", [4, NBLK, 128, 512], F32, kind="ExternalInput").ap()
    cs_d = nc.dram_tensor("cs", [128, T], F32, kind="ExternalInput").ap()
    wq_d = nc.dram_tensor("wqcat", [HIDDEN, 128], F32, kind="ExternalInput").ap()
    wk_d = nc.dram_tensor("wkcat", [HIDDEN, 128], F32, kind="ExternalInput").ap()
    wv_d = nc.dram_tensor("wvT", [HIDDEN, HD], F32, kind="ExternalInput").ap()
    wo_d = nc.dram_tensor("wo2", [128, HIDDEN], F32, kind="ExternalInput").ap()
    istk_d = nc.dram_tensor("istk2", [128, 128], F32, kind="ExternalInput").ap()
    iden_d = nc.dram_tensor("iden", [HD, HD], F32, kind="ExternalInput").ap()
    ones_d = nc.dram_tensor("ones", [128, 32, HD], F32, kind="ExternalInput").ap()
    y_d = nc.dram_tensor("y", [T, HIDDEN], F32, kind="ExternalOutput").ap()

    with tile.TileContext(nc) as tc:
        with tc.tile_pool(name="persist", bufs=1) as sb:
            xT = sb.tile([128, 4, T], F32R)        # 64 KB/part
            cs = sb.tile([128, T], F32)            # 16 KB/part
            wq = sb.tile([128, 4, 128], F32R)
            wk = sb.tile([128, 4, 128], F32R)
            wv = sb.tile([128, 4, HD], F32R)
            wo2 = sb.tile([128, HIDDEN], F32R)     # rows 64-127 zero
            istk2 = sb.tile([128, 128], F32R)      # [[I;I] | 0]
            iden = sb.tile([HD, HD], F32R)
            qT2 = sb.tile([128, T], F32R)          # rows 64-127 zero
            kT2 = sb.tile([128, T], F32R)          # rows 64-127 zero
            va2 = sb.tile([128, 32, 128], F32R)    # cols 64-127 = 1.0
            OT2 = sb.tile([128, T], F32R)          # rows 64-127 = denominators
            recipT = sb.tile([128, 32], F32)

            # small inputs first (weights/tables), then bulk xT pieces
            # (contiguous in DRAM, b-outer so projections start after ~1 MB)
            nc.sync.dma_start(wq[:], wq_d.rearrange("(c p) m -> p c m", p=128).bitcast(F32R))
            nc.sync.dma_start(wk[:], wk_d.rearrange("(c p) m -> p c m", p=128).bitcast(F32R))
            nc.sync.dma_start(wv[:], wv_d.rearrange("(c p) m -> p c m", p=128).bitcast(F32R))
            nc.scalar.dma_start(cs[:], cs_d)
            nc.scalar.dma_start(wo2[:], wo_d.bitcast(F32R))
            nc.scalar.dma_start(istk2[:], istk_d.bitcast(F32R))
            nc.scalar.dma_start(iden[:], iden_d.bitcast(F32R))
            nc.gpsimd.dma_start(va2[:, :, HD:128], ones_d.bitcast(F32R))
            for b in range(NBLK):
                for k in range(4):
                    eng = nc.sync if (k % 2 == 0) else nc.scalar
                    eng.dma_start(xT[:, k, ts(b, 512)],
                                  xT_d[k, b, :, :].bitcast(F32R))

            # ─── Phase P: projections + RoPE + v transposes ───
            with tc.tile_pool(name="pp", bufs=2, space="PSUM") as pp, \
                 tc.tile_pool(name="pf", bufs=2, space="PSUM") as pf, \
                 tc.tile_pool(name="pv", bufs=2, space="PSUM") as pv, \
                 tc.tile_pool(name="ptr", bufs=2, space="PSUM") as ptr, \
                 tc.tile_pool(name="ptmp", bufs=3) as ptmp:

                # warm the exp table set early (overlaps with projections)
                warm = ptmp.tile([1, 16], F32, tag="warm", bufs=1)
                nc.vector.memset(warm[:], 0.0)
                nc.scalar.activation(warm[:], warm[:], EXP, scale=1.0)

                vT = ptmp.tile([HD, T], F32R, tag="vT", bufs=1)
                pending = []
                for b in range(NBLK):
                    pq = pp.tile([128, 512], F32, tag="pp", name="pq")
                    for k in range(4):
                        nc.tensor.matmul(pq[:], wq[:, k, :], xT[:, k, ts(b, 512)],
                                         start=(k == 0), stop=(k == 3))
                    pk = pp.tile([128, 512], F32, tag="pp", name="pk")
                    for k in range(4):
                        nc.tensor.matmul(pk[:], wk[:, k, :], xT[:, k, ts(b, 512)],
                                         start=(k == 0), stop=(k == 3))
                    pv_ = pv.tile([HD, 512], F32, tag="pv", name="pv_")
                    for k in range(4):
                        nc.tensor.matmul(pv_[:], wv[:, k, :], xT[:, k, ts(b, 512)],
                                         start=(k == 0), stop=(k == 3))
                    nc.scalar.copy(vT[:, ts(b, 512)], pv_[:].bitcast(F32))

                    tq = ptmp.tile([128, 512], F32R, tag="tq", name="tq")
                    nc.vector.tensor_tensor(tq[:], pq[:], cs[:, ts(b, 512)],
                                            op=mybir.AluOpType.mult)
                    tk_ = ptmp.tile([128, 512], F32R, tag="tk", name="tk_")
                    nc.vector.tensor_tensor(tk_[:], pk[:], cs[:, ts(b, 512)],
                                            op=mybir.AluOpType.mult)
                    pending.append((tq, tk_, b))
                    if b > 0:
                        _fold(nc, pf, istk2, qT2, kT2, pending)
                    for c in range(4 * b, 4 * b + 4):
                        pt = ptr.tile([128, HD], F32R, tag="ptr", name="pt")
                        nc.tensor.transpose(pt[:], vT[:, ts(c, 128)], iden[:])
                        nc.vector.tensor_copy(va2[:, c, 0:HD], pt[:].bitcast(F32))
                _fold(nc, pf, istk2, qT2, kT2, pending)

            # ─── Phase A+Y: attention quarters with overlapped output proj ───
            with tc.tile_pool(name="po", bufs=2, space="PSUM") as po, \
                 tc.tile_pool(name="psc", bufs=2, space="PSUM") as psc, \
                 tc.tile_pool(name="pa", bufs=3) as pa, \
                 tc.tile_pool(name="yt", bufs=4) as yt, \
                 tc.tile_pool(name="dr", bufs=4, space="DRAM") as dr:

                def emit_quarter(g):
                    """Attention for q columns [1024*g, 1024*(g+1))."""
                    O_ps = po.tile([128, 2, 512], F32, tag="O", name="O_ps", bufs=1)
                    prev = None
                    for c in range(32):
                        s_t = psc.tile([128, 1024], F32, tag="s", name="s_t")
                        for j in range(2):
                            nc.tensor.matmul(s_t[:, ts(j, 512)], kT2[:, ts(c, 128)],
                                             qT2[:, ds(g * 1024 + j * 512, 512)],
                                             start=True, stop=True)
                        a_t = pa.tile([128, 1024], F32R, tag="a", name="a_t")
                        nc.scalar.activation(a_t[:], s_t[:], EXP, scale=0.125)
                        if prev is not None:
                            pc, pa_t = prev
                            for j in range(2):
                                nc.tensor.matmul(O_ps[:, j, :], va2[:, pc, :],
                                                 pa_t[:, ts(j, 512)],
                                                 start=(pc == 0), stop=(pc == 31))
                        prev = (c, a_t)
                    pc, pa_t = prev
                    for j in range(2):
                        nc.tensor.matmul(O_ps[:, j, :], va2[:, pc, :],
                                         pa_t[:, ts(j, 512)],
                                         start=(pc == 0), stop=(pc == 31))
                    # drain: full [128, 1024] copy (rows 64-127 = denominators)
                    nc.vector.tensor_copy(OT2[:, ts(g, 1024)], O_ps[:, :, :])
                    # denominators -> [128, 8] via DRAM roundtrip, reciprocal
                    scr = dr.tile([1, 1024], F32, tag="scr", name="scr")
                    nc.sync.dma_start(scr[:], OT2[64:65, ts(g, 1024)].bitcast(F32))
                    nc.sync.dma_start(
                        recipT[:, ts(g, 8)],
                        scr[0:1, :].rearrange("x (j p) -> (x p) j", p=128))
                    nc.vector.reciprocal(recipT[:, ts(g, 8)], recipT[:, ts(g, 8)])

                def emit_y(g):
                    """Output projection for q-quarter g (8 chunks of 128)."""
                    for i in range(8):
                        qc = g * 8 + i
                        p = po.tile([128, 512], F32, tag="y", name="p_y")
                        nc.tensor.matmul(p[:], OT2[:, ts(qc, 128)], wo2[:],
                                         start=True, stop=True)
                        y_t = yt.tile([128, 512], F32, tag="yt", name="y_t")
                        nc.vector.tensor_scalar_mul(y_t[:], p[:], recipT[:, qc:qc + 1])
                        nc.sync.dma_start(y_d[ts(qc, 128), :], y_t[:])

                for g in range(4):
                    emit_quarter(g)
                    if g > 0:
                        emit_y(g - 1)
                emit_y(3)

    nc.compile()
    return nc


def _host_prep(x, wq, wk, wv, wo, timestamp):
    x2 = np.asarray(x, dtype=np.float32).reshape(T, HIDDEN)
    xT_full = x2.T  # [512, 4096]
    xT = np.ascontiguousarray(
        xT_full.reshape(4, 128, NBLK, 512).transpose(0, 2, 1, 3))

    tsamp = np.asarray(timestamp).reshape(T)
    inv = (1.0 / (np.float32(ROPE_BASE)
                  ** (np.arange(0, HD, 2, dtype=np.float32) / np.float32(HD))))
    freqs = tsamp.astype(np.float32)[:, None] * inv[None, :].astype(np.float32)
    emb = np.concatenate([freqs, freqs], axis=1)
    cs = np.concatenate([np.cos(emb).T, np.sin(emb).T], axis=0)
    cs = np.ascontiguousarray(cs, dtype=np.float32)

    P = np.zeros((HD, HD), dtype=np.float32)
    P[np.arange(32), np.arange(32) + 32] = -1.0
    P[np.arange(32) + 32, np.arange(32)] = 1.0

    istk2 = np.zeros((128, 128), dtype=np.float32)
    istk2[0:64, 0:64] = np.eye(HD)
    istk2[64:128, 0:64] = np.eye(HD)
    iden = np.eye(HD, dtype=np.float32)

    wq = np.asarray(wq, dtype=np.float32)
    wk = np.asarray(wk, dtype=np.float32)
    wv = np.asarray(wv, dtype=np.float32)
    wo = np.asarray(wo, dtype=np.float32)

    in_maps = []
    for h in range(N_HEADS):
        sl = slice(h * HD, (h + 1) * HD)
        wq_h, wk_h, wv_h = wq[sl, :], wk[sl, :], wv[sl, :]
        wo2 = np.zeros((128, HIDDEN), dtype=np.float32)
        wo2[0:HD, :] = wo[:, sl].T
        in_maps.append({
            "xT": xT,
            "cs": cs,
            "wqcat": np.ascontiguousarray(
                np.concatenate([wq_h.T, (P @ wq_h).T], axis=1)),
            "wkcat": np.ascontiguousarray(
                np.concatenate([wk_h.T, (P @ wk_h).T], axis=1)),
            "wvT": np.ascontiguousarray(wv_h.T),
            "wo2": wo2,
            "istk2": istk2,
            "iden": iden,
            "ones": np.ones((128, 32, HD), dtype=np.float32),
        })
    return in_maps


def kernel(x, wq, wk, wv, wo, timestamp):
    if "nc" not in _CACHE:
        _CACHE["nc"] = _build()
    nc = _CACHE["nc"]
    in_maps = _host_prep(x, wq, wk, wv, wo, timestamp)
    r = run_bass_kernel_spmd(nc, in_maps, list(range(N_CORES)))
    y = np.zeros((T, HIDDEN), dtype=np.float64)
    for c in range(N_CORES):
        y += r.results[c]["y"].astype(np.float64)
    return y.astype(np.float32).reshape(1, T, HIDDEN)


# revision 10
# speedup vs baseline: 1.1644x; 1.1644x over previous
"""Trainium2 Bass kernel for nn_NeuralAttention (dense transformer block:
QKV projection + RoPE + softmax attention + output projection).

Sharding: 8 heads -> 8 NeuronCores (tensor parallel, Megatron-style).
Each core computes one head end-to-end from the full input x and produces a
partial output y_h = softmax((q_h k_h^T)/8) v_h @ wo[:, h].T of shape
[4096, 512]; the host sums the 8 partials.

All matmul operands are float32r (tf32-like, ~1e-4 relative error, 1 col/cycle
on the PE). Every contraction is padded to K=128: the TRN2 HAM clock gate
only counts the PE "busy" when the full array is active, so K=64 matmuls run
at 1.2 GHz forever while K=128 runs at 2.4 GHz. Padding the contraction with
zeros (producer side) or multiplying garbage rows by zero weights (consumer
side) doubles the clock at zero cycle cost.

Per-core pipeline (head h):
  xT [512, 4096] (host-transposed x) -> q|rot(q) and k|rot(k) concat-M
  projections -> RoPE via DVE mult with [cos;sin] + PE fold with
  [I;I | 0] -> q'T/k'T [128, 4096] (rows 64-127 zero)
  S.T chunk [128(tk), 512(q)] = kT2_chunk.T @ qT2 (K=128, hi half zeros)
  A.T = exp(S.T/8)  (ACT, PSUM->SBUF, f32r)
  O.T [128, q] += va2_chunk.T @ A.T  (va2 cols 64-127 = 1.0 -> rows 64-127 of
  O.T = softmax denominators), accumulated in PSUM over 32 chunks
  y qchunk [128, 512] = (OT2_chunk.T @ wo2) * recip(sums)  (wo2 rows 64-127
  zero), overlapped with the next q-quarter's attention.
"""

import numpy as np

import concourse.bacc as bacc
import concourse.tile as tile
from concourse import mybir
from concourse.bass import ds, ts
from concourse.bass_utils import run_bass_kernel_spmd

F32 = mybir.dt.float32
F32R = mybir.dt.float32r
EXP = mybir.ActivationFunctionType.Exp

T = 4096
HIDDEN = 512
N_HEADS = 8
HD = 64
N_CORES = 8
NBLK = T // 512
ROPE_BASE = 10000.0

_CACHE = {}


def _fold(nc, pf, istk2, qT2, kT2, pending):
    tq0, tk0, b0 = pending.pop(0)
    pfq = pf.tile([128, 512], F32, tag="pf", name="pfq")
    nc.tensor.matmul(pfq[:], istk2[:], tq0[:], start=True, stop=True)
    nc.scalar.copy(qT2[:, ts(b0, 512)], pfq[:].bitcast(F32))
    pfk = pf.tile([128, 512], F32, tag="pf", name="pfk")
    nc.tensor.matmul(pfk[:], istk2[:], tk0[:], start=True, stop=True)
    nc.scalar.copy(kT2[:, ts(b0, 512)], pfk[:].bitcast(F32))


def _build():
    nc = bacc.Bacc("TRN2", target_bir_lowering=False, debug=False,
                   num_devices=N_CORES)

    xT_d = nc.dram_tensor("xT", [4, NBLK, 128, 512], F32, kind="ExternalInput").ap()
    cs_d = nc.dram_tensor("cs", [128, T], F32, kind="ExternalInput").ap()
    wq_d = nc.dram_tensor("wqcat", [HIDDEN, 128], F32, kind="ExternalInput").ap()
    wk_d = nc.dram_tensor("wkcat", [HIDDEN, 128], F32, kind="ExternalInput").ap()
    wv_d = nc.dram_tensor("wvT", [HIDDEN, HD], F32, kind="ExternalInput").ap()
    wo_d = nc.dram_tensor("wo2", [128, HIDDEN], F32, kind="ExternalInput").ap()
    istk_d = nc.dram_tensor("istk2", [128, 128], F32, kind="ExternalInput").ap()
    iden_d = nc.dram_tensor("iden", [HD, HD], F32, kind="ExternalInput").ap()
    ones_d = nc.dram_tensor("ones", [128, 32, HD], F32, kind="ExternalInput").ap()
    y_d = nc.dram_tensor("y", [T, HIDDEN], F32, kind="ExternalOutput").ap()

    with tile.TileContext(nc) as tc:
        with tc.tile_pool(name="persist", bufs=1) as sb:
            xT = sb.tile([128, 4, T], F32R)        # 64 KB/part
            cs = sb.tile([128, T], F32)            # 16 KB/part
            wq = sb.tile([128, 4, 128], F32R)
            wk = sb.tile([128, 4, 128], F32R)
            wv = sb.tile([128, 4, HD], F32R)
            wo2 = sb.tile([128, HIDDEN], F32R)     # rows 64-127 zero
            istk2 = sb.tile([128, 128], F32R)      # [[I;I] | 0]
            iden = sb.tile([HD, HD], F32R)
            qT2 = sb.tile([128, T], F32R)          # rows 64-127 zero
            kT2 = sb.tile([128, T], F32R)          # rows 64-127 zero
            va2 = sb.tile([128, 32, 128], F32R)    # cols 64-127 = 1.0
            OT2 = sb.tile([128, T], F32R)          # rows 64-127 = denominators
            recipT = sb.tile([128, 32], F32)

            # small inputs first (weights/tables), then bulk xT pieces
            # (contiguous in DRAM, b-outer so projections start after ~1 MB)
            nc.sync.dma_start(wq[:], wq_d.rearrange("(c p) m -> p c m", p=128).bitcast(F32R))
            nc.sync.dma_start(wk[:], wk_d.rearrange("(c p) m -> p c m", p=128).bitcast(F32R))
            nc.sync.dma_start(wv[:], wv_d.rearrange("(c p) m -> p c m", p=128).bitcast(F32R))
            nc.scalar.dma_start(cs[:], cs_d)
            nc.scalar.dma_start(wo2[:], wo_d.bitcast(F32R))
            nc.scalar.dma_start(istk2[:], istk_d.bitcast(F32R))
            nc.scalar.dma_start(iden[:], iden_d.bitcast(F32R))
            nc.gpsimd.dma_start(va2[:, :, HD:128], ones_d.bitcast(F32R))
            for b in range(NBLK):
                for k in range(4):
                    eng = nc.sync if (k % 2 == 0) else nc.scalar
                    eng.dma_start(xT[:, k, ts(b, 512)],
                                  xT_d[k, b, :, :].bitcast(F32R))

            # ─── Phase P: projections + RoPE + v transposes ───
            with tc.tile_pool(name="pp", bufs=2, space="PSUM") as pp, \
                 tc.tile_pool(name="pf", bufs=2, space="PSUM") as pf, \
                 tc.tile_pool(name="pv", bufs=2, space="PSUM") as pv, \
                 tc.tile_pool(name="ptr", bufs=2, space="PSUM") as ptr, \
                 tc.tile_pool(name="ptmp", bufs=3) as ptmp:

                # warm the exp table set early (overlaps with projections)
                warm = ptmp.tile([1, 16], F32, tag="warm", bufs=1)
                nc.vector.memset(warm[:], 0.0)
                nc.scalar.activation(warm[:], warm[:], EXP, scale=1.0)

                vT = ptmp.tile([HD, T], F32R, tag="vT", bufs=1)
                pending = []
                for b in range(NBLK):
                    pq = pp.tile([128, 512], F32, tag="pp", name="pq")
                    for k in range(4):
                        nc.tensor.matmul(pq[:], wq[:, k, :], xT[:, k, ts(b, 512)],
                                         start=(k == 0), stop=(k == 3))
                    pk = pp.tile([128, 512], F32, tag="pp", name="pk")
                    for k in range(4):
                        nc.tensor.matmul(pk[:], wk[:, k, :], xT[:, k, ts(b, 512)],
                                         start=(k == 0), stop=(k == 3))
                    pv_ = pv.tile([HD, 512], F32, tag="pv", name="pv_")
                    for k in range(4):
                        nc.tensor.matmul(pv_[:], wv[:, k, :], xT[:, k, ts(b, 512)],
                                         start=(k == 0), stop=(k == 3))
                    nc.scalar.copy(vT[:, ts(b, 512)], pv_[:].bitcast(F32))

                    tq = ptmp.tile([128, 512], F32R, tag="tq", name="tq")
                    nc.vector.tensor_tensor(tq[:], pq[:], cs[:, ts(b, 512)],
                                            op=mybir.AluOpType.mult)
                    tk_ = ptmp.tile([128, 512], F32R, tag="tk", name="tk_")
                    nc.vector.tensor_tensor(tk_[:], pk[:], cs[:, ts(b, 512)],
                                            op=mybir.AluOpType.mult)
                    pending.append((tq, tk_, b))
                    if b > 0:
                        _fold(nc, pf, istk2, qT2, kT2, pending)
                        for c in range(4 * (b - 1), 4 * b):
                            pt = ptr.tile([128, HD], F32R, tag="ptr", name="pt")
                            nc.tensor.transpose(pt[:], vT[:, ts(c, 128)], iden[:])
                            nc.vector.tensor_copy(va2[:, c, 0:HD], pt[:].bitcast(F32))
                _fold(nc, pf, istk2, qT2, kT2, pending)
                for c in range(28, 32):
                    pt = ptr.tile([128, HD], F32R, tag="ptr", name="pt")
                    nc.tensor.transpose(pt[:], vT[:, ts(c, 128)], iden[:])
                    nc.vector.tensor_copy(va2[:, c, 0:HD], pt[:].bitcast(F32))

            # ─── Phase A+Y: attention quarters with overlapped output proj ───
            with tc.tile_pool(name="po", bufs=2, space="PSUM") as po, \
                 tc.tile_pool(name="psc", bufs=2, space="PSUM") as psc, \
                 tc.tile_pool(name="pa", bufs=3) as pa, \
                 tc.tile_pool(name="yt", bufs=4) as yt, \
                 tc.tile_pool(name="dr", bufs=4, space="DRAM") as dr:

                def emit_quarter(g):
                    """Attention for q columns [1024*g, 1024*(g+1))."""
                    O_ps = po.tile([128, 2, 512], F32, tag="O", name="O_ps", bufs=1)
                    prev = None
                    for c in range(32):
                        s_t = psc.tile([128, 1024], F32, tag="s", name="s_t")
                        for j in range(2):
                            nc.tensor.matmul(s_t[:, ts(j, 512)], kT2[:, ts(c, 128)],
                                             qT2[:, ds(g * 1024 + j * 512, 512)],
                                             start=True, stop=True)
                        a_t = pa.tile([128, 1024], F32R, tag="a", name="a_t")
                        nc.scalar.activation(a_t[:], s_t[:], EXP, scale=0.125)
                        if prev is not None:
                            pc, pa_t = prev
                            for j in range(2):
                                nc.tensor.matmul(O_ps[:, j, :], va2[:, pc, :],
                                                 pa_t[:, ts(j, 512)],
                                                 start=(pc == 0), stop=(pc == 31))
                        prev = (c, a_t)
                    pc, pa_t = prev
                    for j in range(2):
                        nc.tensor.matmul(O_ps[:, j, :], va2[:, pc, :],
                                         pa_t[:, ts(j, 512)],
                                         start=(pc == 0), stop=(pc == 31))
                    # drain: full [128, 1024] copy (rows 64-127 = denominators)
                    nc.vector.tensor_copy(OT2[:, ts(g, 1024)], O_ps[:, :, :])
                    # denominators -> [128, 8] via DRAM roundtrip, reciprocal
                    scr = dr.tile([1, 1024], F32, tag="scr", name="scr")
                    nc.sync.dma_start(scr[:], OT2[64:65, ts(g, 1024)].bitcast(F32))
                    nc.sync.dma_start(
                        recipT[:, ts(g, 8)],
                        scr[0:1, :].rearrange("x (j p) -> (x p) j", p=128))
                    nc.vector.reciprocal(recipT[:, ts(g, 8)], recipT[:, ts(g, 8)])

                def emit_y(g):
                    """Output projection for q-quarter g (8 chunks of 128)."""
                    for i in range(8):
                        qc = g * 8 + i
                        p = po.tile([128, 512], F32, tag="y", name="p_y")
                        nc.tensor.matmul(p[:], OT2[:, ts(qc, 128)], wo2[:],
                                         start=True, stop=True)
                        y_t = yt.tile([128, 512], F32, tag="yt", name="y_t")
                        nc.vector.tensor_scalar_mul(y_t[:], p[:], recipT[:, qc:qc + 1])
                        nc.sync.dma_start(y_d[ts(qc, 128), :], y_t[:])

                for g in range(4):
                    emit_quarter(g)
                    if g > 0:
                        emit_y(g - 1)
                emit_y(3)

    nc.compile()
    return nc


def _host_prep(x, wq, wk, wv, wo, timestamp):
    x2 = np.asarray(x, dtype=np.float32).reshape(T, HIDDEN)
    xT_full = x2.T  # [512, 4096]
    xT = np.ascontiguousarray(
        xT_full.reshape(4, 128, NBLK, 512).transpose(0, 2, 1, 3))

    tsamp = np.asarray(timestamp).reshape(T)
    inv = (1.0 / (np.float32(ROPE_BASE)
                  ** (np.arange(0, HD, 2, dtype=np.float32) / np.float32(HD))))
    freqs = tsamp.astype(np.float32)[:, None] * inv[None, :].astype(np.float32)
    emb = np.concatenate([freqs, freqs], axis=1)
    cs = np.concatenate([np.cos(emb).T, np.sin(emb).T], axis=0)
    cs = np.ascontiguousarray(cs, dtype=np.float32)

    P = np.zeros((HD, HD), dtype=np.float32)
    P[np.arange(32), np.arange(32) + 32] = -1.0
    P[np.arange(32) + 32, np.arange(32)] = 1.0

    istk2 = np.zeros((128, 128), dtype=np.float32)
    istk2[0:64, 0:64] = np.eye(HD)
    istk2[64:128, 0:64] = np.eye(HD)
    iden = np.eye(HD, dtype=np.float32)

    wq = np.asarray(wq, dtype=np.float32)
    wk = np.asarray(wk, dtype=np.float32)
    wv = np.asarray(wv, dtype=np.float32)
    wo = np.asarray(wo, dtype=np.float32)

    in_maps = []
    for h in range(N_HEADS):
        sl = slice(h * HD, (h + 1) * HD)
        wq_h, wk_h, wv_h = wq[sl, :], wk[sl, :], wv[sl, :]
        wo2 = np.zeros((128, HIDDEN), dtype=np.float32)
        wo2[0:HD, :] = wo[:, sl].T
        in_maps.append({
            "xT": xT,
            "cs": cs,
            "wqcat": np.ascontiguousarray(
                np.concatenate([wq_h.T, (P @ wq_h).T], axis=1)),
            "wkcat": np.ascontiguousarray(
                np.concatenate([wk_h.T, (P @ wk_h).T], axis=1)),
            "wvT": np.ascontiguousarray(wv_h.T),
            "wo2": wo2,
            "istk2": istk2,
            "iden": iden,
            "ones": np.ones((128, 32, HD), dtype=np.float32),
        })
    return in_maps


def kernel(x, wq, wk, wv, wo, timestamp):
    if "nc" not in _CACHE:
        _CACHE["nc"] = _build()
    nc = _CACHE["nc"]
    in_maps = _host_prep(x, wq, wk, wv, wo, timestamp)
    r = run_bass_kernel_spmd(nc, in_maps, list(range(N_CORES)))
    y = np.zeros((T, HIDDEN), dtype=np.float64)
    for c in range(N_CORES):
        y += r.results[c]["y"].astype(np.float64)
    return y.astype(np.float32).reshape(1, T, HIDDEN)
